# revision 14
# baseline (speedup 1.0000x reference)
"""Causal self-attention with rotary embeddings on 8 Trainium2 NeuronCores.

Sharding: 2-way data-parallel over batch x 4-way tensor-parallel over heads.
Core c handles batch (c // 4) and heads [4*(c%4), 4*(c%4)+4), as two
head-pairs p=0,1 (a pair = 2 heads packed into one 128-partition tile,
head-even in partitions 0-63, head-odd in 64-127).

Per-core pipeline (T=2048 tokens, one batch):
  phase1(p): qkv for pair p (full-rate 128-contraction matmuls), rotary via
    pair-swap permutation matmul + DVE muls, V transposed to t-major Vaug
    tiles [k, d|ones] (ones column folds the softmax denominator into PV).
  attn(p): per q-chunk c (512) and k-block j (128): scores^T = K_blk @ Q^T
    for both heads CONCURRENTLY via PE row-tiling (head-even uses array rows
    0-63, head-odd rows 64-127); one exp (ScalarE) covers both heads; causal
    mask on diagonal blocks; PV accumulates exact-length streams (no
    padding); normalize per chunk via reciprocal of the ones-row.
  proj: y = sum_p Yn_p^T @ wp_p, interleaved with attn(p=1) per chunk.

Program order interleaves phase1(p=1) with attn(p=0) chunks and proj waves
with attn(p=1) chunks so the Tile scheduler keeps the PE dense (HAM-warm).
All matmul inputs fp16 (1 cyc/row), fp32 PSUM accumulation.
"""

import numpy as np

B, T, C, H = 2, 2048, 1024, 16
HD = C // H            # 64
N_CORES = 8
HPC = 4                # heads per core
NPAIR = 2              # head pairs per core
TC = 512               # t-chunk for phase 1
NTC = T // TC          # 4
KB = 128               # k-block size
NKB = T // KB          # 16
QC = 512               # q-chunk for attention
NQC = T // QC          # 4
CCH = C // 128         # 8 contraction chunks

_CACHE = {}


def _build_bass(debug=False):
    import concourse.bacc as bacc
    import concourse.mybir as mybir
    import concourse.tile as tile
    from concourse.masks import make_identity, make_upper_triangular

    f16 = mybir.dt.float16
    f32 = mybir.dt.float32
    EXP = mybir.ActivationFunctionType.Exp
    MULT = mybir.AluOpType.mult

    nc = bacc.Bacc()

    xT = nc.dram_tensor("xT", [C, T], f16, kind="ExternalInput")
    wqkv = nc.dram_tensor("wqkv", [C, 6 * 128], f16, kind="ExternalInput")
    wp = nc.dram_tensor("wp", [2 * 128, C], f16, kind="ExternalInput")
    cos_e = nc.dram_tensor("cos_e", [128, T], f16, kind="ExternalInput")
    sin_e = nc.dram_tensor("sin_e", [128, T], f16, kind="ExternalInput")
    pswap = nc.dram_tensor("pswap", [128, 128], f16, kind="ExternalInput")
    y = nc.dram_tensor("y", [T, C], f16, kind="ExternalOutput")

    if debug:
        dbg_qrot = nc.dram_tensor("dbg_qrot", [128, T], f16,
                                  kind="ExternalOutput")
        dbg_krot = nc.dram_tensor("dbg_krot", [128, T], f16,
                                  kind="ExternalOutput")
        dbg_vaug = nc.dram_tensor("dbg_vaug", [128, NKB * 130], f16,
                                  kind="ExternalOutput")
        dbg_yn = nc.dram_tensor("dbg_yn", [128, 2 * T], f16,
                                kind="ExternalOutput")

    with tile.TileContext(nc) as tc:
        with (
            tc.tile_pool(name="const", bufs=1) as const,
            tc.tile_pool(name="persist", bufs=1) as persist,
            tc.tile_pool(name="xbuf", bufs=1) as xbuf,
            tc.tile_pool(name="stream", bufs=2) as stream,
            tc.tile_pool(name="pts", bufs=8) as pts,
            tc.tile_pool(name="psum", bufs=1, space="PSUM") as psum,
        ):
            # ---- constants + x, DMA-ordered for earliest first matmul ----
            x_sb = xbuf.tile([128, NTC, CCH, TC], f16)
            xT_r = xT.rearrange("(cc p) (i t) -> p i cc t", p=128, t=TC)
            wqkv_sb = const.tile([128, CCH, 6 * 128], f16)
            wqkv_r = wqkv.rearrange("(cc p) (g j) -> p cc g j", p=128, j=128)
            wqkv_v = wqkv_sb.rearrange("p cc (g j) -> p cc g j", j=128)
            # first qkv chain needs g0 + x0 per-cc slices: queue those first
            nc.sync.dma_start(out=wqkv_v[:, :, 0], in_=wqkv_r[:, :, 0])
            for cc in range(CCH):
                nc.sync.dma_start(out=x_sb[:, 0, cc], in_=xT_r[:, 0, cc])
            pswap_sb = const.tile([128, 128], f16)
            nc.sync.dma_start(out=pswap_sb, in_=pswap[:, :])
            for g in [2, 4]:  # rest of pair-0 groups
                nc.sync.dma_start(out=wqkv_v[:, :, g], in_=wqkv_r[:, :, g])
            cos_sb = const.tile([128, T], f16)
            nc.sync.dma_start(out=cos_sb, in_=cos_e[:, :])
            sin_sb = const.tile([128, T], f16)
            nc.sync.dma_start(out=sin_sb, in_=sin_e[:, :])
            for i in range(1, NTC):
                nc.sync.dma_start(out=x_sb[:, i], in_=xT_r[:, i])
            for g in [1, 3, 5]:  # pair-1 groups
                nc.sync.dma_start(out=wqkv_v[:, :, g], in_=wqkv_r[:, :, g])
            wp_sb = const.tile([128, 2, C], f16)
            nc.sync.dma_start(
                out=wp_sb, in_=wp.rearrange("(p d) c -> d p c", p=2))
            ident = const.tile([128, 128], f16)
            make_identity(nc, ident)
            # mask[k, q] = 1 where q >= k (keep), 0 where q < k
            mask_ut = const.tile([128, 128], f16)
            make_upper_triangular(nc, mask_ut, val=1.0, diag=True)

            # ---- persistent per-pair tensors ----
            QrotT = [persist.tile([128, T], f16, name=f"qrot{p}")
                     for p in range(NPAIR)]
            KrotT = [persist.tile([128, T], f16, name=f"krot{p}")
                     for p in range(NPAIR)]
            # V t-major per k-block: [V_even(64) | ones | V_odd(64) | ones]
            Vaug = [persist.tile([128, NKB, 130], f16, name=f"vaug{p}")
                    for p in range(NPAIR)]
            Yn = [persist.tile([128, T], f16, name=f"yn{p}")
                  for p in range(NPAIR)]
            for p in range(NPAIR):
                ones_cols = Vaug[p].rearrange(
                    "q J (h x) -> q J h x", x=65)[:, :, :, 64]
                nc.gpsimd.memset(ones_cols, 1.0)

            def phase1_chunk(p, i):
                """qkv + rotary + V transpose for pair p, t-chunk i."""
                ts = slice(i * TC, (i + 1) * TC)
                for g in range(3):  # Q, K, V
                    gc = (g * 2 + p) * 128
                    acc = psum.tile([128, 2, TC], f32, tag="st", bufs=2, name="acc")
                    for cc in range(CCH):
                        nc.tensor.matmul(
                            acc[:, 0, :], wqkv_sb[:, cc, gc:gc + 128],
                            x_sb[:, i, cc, :],
                            start=(cc == 0), stop=(cc == CCH - 1))
                    if g < 2:  # rotary for Q or K
                        dst = (QrotT if g == 0 else KrotT)[p]
                        graw = stream.tile([128, TC], f16, tag="graw")
                        nc.vector.tensor_copy(graw, acc[:, 0, :])
                        nc.tensor.matmul(acc[:, 1, :], pswap_sb, graw,
                                         start=True, stop=True)
                        t1 = stream.tile([128, TC], f16, tag="t1")
                        nc.vector.tensor_mul(t1, graw, cos_sb[:, ts])
                        t2 = stream.tile([128, TC], f16, tag="t2")
                        nc.vector.tensor_mul(t2, acc[:, 1, :], sin_sb[:, ts])
                        nc.vector.tensor_add(dst[:, ts], t1, t2)
                    else:  # V -> t-major
                        vtmp = stream.tile([128, TC], f16, tag="vtmp")
                        nc.vector.tensor_copy(vtmp, acc[:, 0, :])
                        for q in range(TC // 128):
                            J = i * (TC // 128) + q
                            vt = psum.tile([128, 128], f16, tag="yps",
                                           bufs=2, name="vt")
                            nc.tensor.transpose(
                                vt, vtmp[:, q * 128:(q + 1) * 128], ident)
                            vdst = Vaug[p].rearrange(
                                "k J (h x) -> k J h x", x=65)[:, J, :, 0:64]
                            nc.vector.tensor_copy(
                                vdst, vt.rearrange("k (h x) -> k h x", h=2))

            def attn_chunk(p, c):
                """scores+exp+mask+PV+normalize for pair p, q-chunk c."""
                yps = psum.tile([128, 2, QC], f32, tag="yps", bufs=2, name="yps")
                jmax = 4 * c + 3
                for j in range(jmax + 1):
                    qs = max(c * QC, j * KB)
                    n = (c + 1) * QC - qs
                    st = psum.tile([128, 2, QC], f32, tag="st", bufs=2, name="st")
                    for h in range(2):
                        hs = slice(h * 64, (h + 1) * 64)
                        nc.tensor.matmul(
                            st[:, h, 0:n],
                            KrotT[p][hs, j * KB:j * KB + 128],
                            QrotT[p][hs, qs:qs + n],
                            start=True, stop=True)
                    pt = pts.tile([128, 2, n], f16, tag=f"pt{n}")
                    nc.scalar.activation(pt, st[:, :, 0:n], EXP)
                    if j >= 4 * c:  # diagonal block: causal mask
                        nc.vector.tensor_mul(
                            pt[:, 0, 0:128], pt[:, 0, 0:128], mask_ut)
                        nc.vector.tensor_mul(
                            pt[:, 1, 0:128], pt[:, 1, 0:128], mask_ut)
                    for h in range(2):
                        nc.tensor.matmul(
                            yps[0:65, h, QC - n:QC],
                            Vaug[p][:, j, h * 65:(h + 1) * 65],
                            pt[:, h, :],
                            start=(j == 0), stop=(j == jmax),
                            skip_group_check=True)
                # normalize: rows 0-63 divided by the ones-row (64)
                dsb = stream.tile([1, 2, QC], f32, tag="dsb")
                nc.scalar.copy(dsb, yps[64:65, :, :])
                recip = stream.tile([1, 2, QC], f32, tag="recip")
                nc.vector.reciprocal_approx_fast(out=recip, in_=dsb)
                bc = stream.tile([128, 2, QC], f32, tag="bc")
                nc.gpsimd.partition_broadcast(bc[0:64], recip[0:1])
                cs = slice(c * QC, (c + 1) * QC)
                nc.vector.tensor_tensor(
                    out=Yn[p][0:64, cs], in0=yps[0:64, 0, :],
                    in1=bc[0:64, 0, :], op=MULT)
                ytmp = stream.tile([128, QC], f16, tag="ytmp")
                nc.vector.tensor_tensor(
                    out=ytmp[0:64, :], in0=yps[0:64, 1, :],
                    in1=bc[0:64, 1, :], op=MULT)
                nc.sync.dma_start(out=Yn[p][64:128, cs], in_=ytmp[0:64, :])

            def proj_tt(tt):
                """output projection for token block tt (128 tokens)."""
                pout = psum.tile([128, 2, 512], f32, tag="yps", bufs=2, name="pout")
                for half in range(2):
                    hc = slice(half * 512, (half + 1) * 512)
                    for p in range(NPAIR):
                        nc.tensor.matmul(
                            pout[:, half, :],
                            Yn[p][:, tt * 128:(tt + 1) * 128],
                            wp_sb[:, p, hc],
                            start=(p == 0), stop=(p == NPAIR - 1))
                yout = stream.tile([128, 2, 512], f16, tag="yout")
                if tt % 2 == 0:
                    nc.vector.tensor_copy(yout, pout)
                else:
                    nc.scalar.copy(yout, pout)
                nc.sync.dma_start(
                    out=y[tt * 128:(tt + 1) * 128, :],
                    in_=yout.rearrange("t h c -> t (h c)"))

            # ---- program order: interleave for engine overlap ----
            # attn(p, c) needs only ph1(p, 0..c), so exp starts early; ph1
            # chunks and proj waves fill PE while ScalarE runs exp.  attn
            # chunk order (0, 2, 3, 1) puts a small chunk last to shrink
            # the ScalarE-paced tail.
            def proj_wave(c):
                for tt in range(4 * c, 4 * c + 4):
                    proj_tt(tt)

            for c in range(NQC):
                phase1_chunk(0, c)
                attn_chunk(0, c)
            for c in range(NQC):
                phase1_chunk(1, c)
                attn_chunk(1, c)
                proj_wave(c)

            if debug:
                nc.sync.dma_start(out=dbg_qrot[:, :], in_=QrotT[0])
                nc.sync.dma_start(out=dbg_krot[:, :], in_=KrotT[0])
                nc.sync.dma_start(
                    out=dbg_vaug[:, :],
                    in_=Vaug[0].rearrange("k J x -> k (J x)"))
                nc.sync.dma_start(out=dbg_yn[:, 0:T], in_=Yn[0])
                nc.sync.dma_start(out=dbg_yn[:, T:2 * T], in_=Yn[1])

    nc.finalize()
    return nc


def _host_prep(x, cos, sin, w_attn, b_attn, w_proj):
    """Shared + per-core input arrays (fp16)."""
    x2 = np.asarray(x, dtype=np.float32).reshape(B * T, C)
    xT16 = np.ascontiguousarray(x2.T).astype(np.float16)  # [C, B*T]

    cos = np.asarray(cos, dtype=np.float32)
    sin = np.asarray(sin, dtype=np.float32)
    d = np.arange(128) % 64
    freq_i = d // 2
    sign = np.where(d % 2 == 0, -1.0, 1.0).astype(np.float32)
    cos_exp = cos[:, freq_i].T.astype(np.float16)           # [128, T]
    sin_exp = (sign[:, None] * sin[:, freq_i].T).astype(np.float16)

    pswap = np.zeros((128, 128), dtype=np.float16)
    idx = np.arange(128)
    pswap[idx ^ 1, idx] = 1.0

    w_attn = np.asarray(w_attn, dtype=np.float32)
    w_proj = np.asarray(w_proj, dtype=np.float32)
    scale = 1.0 / np.sqrt(HD)

    per_group = []
    for hg in range(4):
        cols = []
        for g in range(3):          # q, k, v
            for p in range(NPAIR):  # head pair within group
                for hh in range(2):
                    hglob = hg * HPC + p * 2 + hh
                    blk = w_attn[:, g * C + hglob * HD:
                                 g * C + (hglob + 1) * HD]
                    if g == 0:
                        blk = blk * scale
                    cols.append(blk)
        # reorder to [qp0, qp1, kp0, kp1, vp0, vp1] each 128 wide
        w_stack = np.concatenate(cols, axis=1).astype(np.float16)
        wp_m = w_proj[hg * HPC * HD:(hg + 1) * HPC * HD, :].astype(np.float16)
        per_group.append((w_stack, wp_m))
    return xT16, cos_exp, sin_exp, pswap, per_group


def kernel(x, cos, sin, w_attn, b_attn, w_proj, b_proj):
    from concourse.bass_utils import run_bass_kernel_spmd

    b_attn = np.asarray(b_attn, dtype=np.float32)
    assert not np.any(b_attn), "nonzero b_attn not supported by this kernel"

    xT16, cos_exp, sin_exp, pswap, per_group = _host_prep(
        x, cos, sin, w_attn, b_attn, w_proj)

    if "nc" not in _CACHE:
        _CACHE["nc"] = _build_bass()
    nc = _CACHE["nc"]

    in_maps = []
    for m in range(N_CORES):
        bg, hg = m // 4, m % 4
        w_stack, wp_m = per_group[hg]
        in_maps.append({
            "xT": np.ascontiguousarray(xT16[:, bg * T:(bg + 1) * T]),
            "wqkv": w_stack, "wp": wp_m,
            "cos_e": cos_exp, "sin_e": sin_exp, "pswap": pswap,
        })

    res = run_bass_kernel_spmd(nc, in_maps, core_ids=list(range(N_CORES)))
    _CACHE["last_result"] = res

    out = np.zeros((B, T, C), dtype=np.float64)
    for m in range(N_CORES):
        bg = m // 4
        out[bg] += res.results[m]["y"].astype(np.float64)
    out += np.asarray(b_proj, dtype=np.float64)[None, None, :]
    return out.astype(np.float32)
